# revision 2
# baseline (speedup 1.0000x reference)
"""Trainium2 Bass kernel for nn_Net_3152505995417 (gnn_message_passing).

Closed-form reformulation: with T the incidence matrix of a simple graph,
  node conv:  (T diag(d) T^T) * adj_v  ==  A with A[i,j] = d[edge(i,j)], 0 diag
  edge conv:  M = (T^T diag(dv) T) * adj_e has M[e,f] = dv[shared node],
              col-max(f=(k,l)) = max(dv[k], dv[l], 0)   (complete graph),
              row e=(i,j) of (M/colmax) @ G = dv_i*(S_i - Gn_e) + dv_j*(S_j - Gn_e)
              with Gn = G / (colmax + eps), S = T @ Gn.
So the E x E matrix is never materialized: everything lives in a dense
[N, N] node-pair layout (both (i,j) and (j,i) slots carry edge {i,j};
diagonal slots are zero). All gathers/scatters become row/column
broadcasts and row-sums of [116, 116] tiles.

The full model (2 node convs + 1 edge conv + classifier head) runs on
each of the 8 NeuronCores (work is tiny; replication avoids collective
latency); core 0's output is returned.
"""

import numpy as np

N = 116
E = N * (N - 1) // 2
HID = 64
EDIM = 5
OUT = 4
ENC = HID + N // 2
EPS = 1e-10

_CACHE = {}


def _split_excess_waits(nc, mybir, max_waits=1):
    """Workaround: this walrus build accepts only one sync-wait per engine
    instruction (setupSyncWait: "Too many sync wait commands"). Move excess
    waits onto chained NoOps on the same engine immediately before the
    instruction; sequencer semantics are unchanged."""
    for fn in nc.m.functions:
        for blk in fn.blocks:
            insts = blk.instructions
            new, changed = [], False
            for ins in insts:
                si = ins.sync_info
                waits = list(si.on_wait) if si is not None else []
                if len(waits) > max_waits:
                    while len(waits) > max_waits:
                        chunk, waits = waits[:1], waits[1:]
                        nop = mybir.InstNoOp(
                            name=nc.get_next_instruction_name(),
                            engine=ins.engine,
                            sync_info=mybir.SyncInfo(on_wait=chunk, on_update=[]),
                            bass_nofuse=True,
                        )
                        new.append(nop)
                    si.on_wait = waits
                    changed = True
                new.append(ins)
            if changed:
                blk.instructions = new


def _build():
    import concourse.bass as bass
    import concourse.tile as tile
    from concourse import mybir

    f32 = mybir.dt.float32
    A = mybir.AluOpType
    Relu = mybir.ActivationFunctionType.Relu

    nc = bass.Bass("TRN2", target_bir_lowering=False, num_devices=8)

    dp = nc.declare_dram_parameter
    ea_d = dp("ea", [N, EDIM * N], f32, isOutput=False)      # k-major: f = k*N + j
    encT_d = dp("encT", [ENC, N], f32, isOutput=False)
    Wenc_d = dp("Wenc", [ENC, HID], f32, isOutput=False)
    W1_d = dp("W1", [HID, HID], f32, isOutput=False)
    W2_d = dp("W2", [HID, HID], f32, isOutput=False)
    Wl_d = dp("Wl", [HID, OUT], f32, isOutput=False)
    benc_d = dp("benc", [HID, 1], f32, isOutput=False)
    b1_d = dp("b1", [HID, 1], f32, isOutput=False)
    b2_d = dp("b2", [HID, 1], f32, isOutput=False)
    bl_d = dp("bl", [OUT, 1], f32, isOutput=False)
    peT_d = dp("peT", [HID, 1], f32, isOutput=False)
    svec_d = dp("svec", [1, 40], f32, isOutput=False)        # p1|p2|be|We.flat
    mask_d = dp("mask", [N, N], f32, isOutput=False)         # 1 - eye
    out_d = dp("out", [OUT, 1], f32, isOutput=True)

    with tile.TileContext(nc) as tc:
        with (
            tc.tile_pool(name="sb", bufs=1) as sb,
            tc.tile_pool(name="pm", bufs=2) as pm,
            tc.tile_pool(name="ps", bufs=4, space="PSUM") as ps,
        ):
            # ---- loads ----
            def load(name, shape, src):
                t = sb.tile(shape, f32, tag=name)
                nc.sync.dma_start(out=t[:], in_=src[:])
                return t

            ea = load("ea", [N, EDIM * N], ea_d)
            encT = load("encT", [ENC, N], encT_d)
            Wenc = load("Wenc", [ENC, HID], Wenc_d)
            W1 = load("W1", [HID, HID], W1_d)
            W2 = load("W2", [HID, HID], W2_d)
            Wl = load("Wl", [HID, OUT], Wl_d)
            benc = load("benc", [HID, 1], benc_d)
            b1 = load("b1", [HID, 1], b1_d)
            b2 = load("b2", [HID, 1], b2_d)
            bl = load("bl", [OUT, 1], bl_d)
            peT = load("peT", [HID, 1], peT_d)
            svec = load("svec", [1, 40], svec_d)
            mask = load("mask", [N, N], mask_d)

            ones_row = sb.tile([1, N], f32, tag="ones_row")
            nc.vector.memset(ones_row[:], 1.0)
            ones_col = sb.tile([N, 1], f32, tag="ones_col")
            nc.vector.memset(ones_col[:], 1.0)

            # ---- broadcast the small row-vector params to all partitions ----
            svecB_ps = ps.tile([N, 40], f32, tag="ps")
            nc.tensor.matmul(svecB_ps[:], ones_row[:], svec[:], start=True, stop=True)
            svecB = sb.tile([N, 40], f32, tag="svecB")
            nc.vector.tensor_copy(svecB[:], svecB_ps[:])
            p1B = svecB[:, 0:5]
            p2B = svecB[:, 5:10]
            beB = svecB[:, 10:15]
            # We[k, m] at column 15 + k*5 + m

            # ---- x = enc @ W_enc + b_enc  (kept transposed: [HID, N]) ----
            xT_ps = ps.tile([HID, N], f32, tag="ps")
            nc.tensor.matmul(xT_ps[:], Wenc[:], encT[:], start=True, stop=True)
            xT = sb.tile([HID, N], f32, tag="xT")
            nc.vector.tensor_scalar_add(xT[:], xT_ps[:], benc[:, 0:1])

            # ---- A1 = d1 (dense pair layout; diag slots already zero) ----
            d1 = sb.tile([N, N], f32, tag="d1")
            nc.vector.tensor_scalar_mul(d1[:], ea[:, 0:N], p1B[:, 0:1])
            for k in range(1, EDIM):
                nc.vector.scalar_tensor_tensor(
                    d1[:], ea[:, k * N:(k + 1) * N], p1B[:, k:k + 1], d1[:],
                    A.mult, A.add,
                )

            # ---- node conv 1: x1T = relu((A1 @ (x @ W1) + b1)^T) ----
            xW1_ps = ps.tile([N, HID], f32, tag="ps")
            nc.tensor.matmul(xW1_ps[:], xT[:], W1[:], start=True, stop=True)
            xW1 = sb.tile([N, HID], f32, tag="xW1")
            nc.vector.tensor_copy(xW1[:], xW1_ps[:])
            x1T_ps = ps.tile([HID, N], f32, tag="ps")
            nc.tensor.matmul(x1T_ps[:], xW1[:], d1[:], start=True, stop=True)
            x1T = sb.tile([HID, N], f32, tag="x1T")
            nc.scalar.activation(x1T[:], x1T_ps[:], Relu, bias=b1[:, 0:1])

            # ---- dv = x1 @ pe^T, as row [1,N] and column [N,1] ----
            dvr_ps = ps.tile([1, N], f32, tag="ps")
            nc.tensor.matmul(dvr_ps[:], peT[:], x1T[:], start=True, stop=True)
            dv_row = sb.tile([1, N], f32, tag="dv_row")
            nc.vector.tensor_copy(dv_row[:], dvr_ps[:])
            dvT_ps = ps.tile([N, 1], f32, tag="ps")
            nc.tensor.matmul(dvT_ps[:], x1T[:], peT[:], start=True, stop=True)
            dvT = sb.tile([N, 1], f32, tag="dvT")
            nc.vector.tensor_copy(dvT[:], dvT_ps[:])
            dvROW_ps = ps.tile([N, N], f32, tag="ps")
            nc.tensor.matmul(dvROW_ps[:], ones_row[:], dv_row[:], start=True, stop=True)
            dvROW = sb.tile([N, N], f32, tag="dvROW")
            nc.vector.tensor_copy(dvROW[:], dvROW_ps[:])

            # negsumdv[i,j] = -(dv_i + dv_j);  recip = 1/(max(dv_i,dv_j,0)+eps)
            negsumdv = sb.tile([N, N], f32, tag="negsumdv")
            nc.vector.tensor_scalar(
                negsumdv[:], dvROW[:], dvT[:, 0:1], -1.0, A.add, A.mult
            )
            recip = sb.tile([N, N], f32, tag="recip")
            nc.vector.tensor_scalar(
                recip[:], dvROW[:], dvT[:, 0:1], 0.0, A.max, A.max
            )
            nc.vector.tensor_scalar_add(recip[:], recip[:], EPS)
            nc.vector.reciprocal(recip[:], recip[:])

            # ---- edge conv ----
            eR = sb.tile([N, EDIM * N], f32, tag="eR")
            nc.scalar.activation(eR[:], ea[:], Relu)
            e2 = sb.tile([N, EDIM * N], f32, tag="e2")  # m-major: f = m*N + j

            for m in range(EDIM):
                Gm = pm.tile([N, N], f32, tag="Gm")
                nc.vector.tensor_scalar_mul(
                    Gm[:], eR[:, 0:N], svecB[:, 15 + m:15 + m + 1]
                )
                for k in range(1, EDIM):
                    nc.vector.scalar_tensor_tensor(
                        Gm[:], eR[:, k * N:(k + 1) * N],
                        svecB[:, 15 + k * 5 + m:15 + k * 5 + m + 1], Gm[:],
                        A.mult, A.add,
                    )
                Gn = pm.tile([N, N], f32, tag="Gn")
                S_col = pm.tile([N, 1], f32, tag="S_col")
                nc.vector.scalar_tensor_tensor(
                    Gn[:], Gm[:], 0.0, recip[:], A.add, A.mult,
                    accum_out=S_col[:],
                )
                S_row_ps = ps.tile([1, N], f32, tag="ps")
                nc.tensor.matmul(S_row_ps[:], ones_col[:], Gn[:], start=True, stop=True)
                u = pm.tile([1, N], f32, tag="u")
                nc.vector.tensor_tensor(u[:], dv_row[:], S_row_ps[:], A.mult)
                U_ps = ps.tile([N, N], f32, tag="ps")
                nc.tensor.matmul(U_ps[:], ones_row[:], u[:], start=True, stop=True)
                t1b = pm.tile([N, 1], f32, tag="t1b")
                nc.vector.scalar_tensor_tensor(
                    t1b[:], S_col[:], dvT[:, 0:1], beB[:, m:m + 1], A.mult, A.add
                )
                q = pm.tile([N, N], f32, tag="q")
                nc.vector.tensor_tensor(q[:], Gn[:], negsumdv[:], A.mult)
                z = pm.tile([N, N], f32, tag="z")
                nc.vector.tensor_tensor(z[:], q[:], U_ps[:], A.add)
                nc.scalar.activation(
                    e2[:, m * N:(m + 1) * N], z[:], Relu, bias=t1b[:, 0:1]
                )

            # ---- A2 = (e2 @ p2^T) * mask ----
            d2 = sb.tile([N, N], f32, tag="d2")
            nc.vector.tensor_scalar_mul(d2[:], e2[:, 0:N], p2B[:, 0:1])
            for m in range(1, EDIM):
                nc.vector.scalar_tensor_tensor(
                    d2[:], e2[:, m * N:(m + 1) * N], p2B[:, m:m + 1], d2[:],
                    A.mult, A.add,
                )
            A2 = sb.tile([N, N], f32, tag="A2")
            nc.vector.tensor_tensor(A2[:], d2[:], mask[:], A.mult)

            # ---- node conv 2 (no relu) + mean pool + head ----
            xW2_ps = ps.tile([N, HID], f32, tag="ps")
            nc.tensor.matmul(xW2_ps[:], x1T[:], W2[:], start=True, stop=True)
            xW2 = sb.tile([N, HID], f32, tag="xW2")
            nc.vector.tensor_copy(xW2[:], xW2_ps[:])
            x2T_ps = ps.tile([HID, N], f32, tag="ps")
            nc.tensor.matmul(x2T_ps[:], xW2[:], A2[:], start=True, stop=True)
            red = sb.tile([HID, 1], f32, tag="red")
            nc.vector.tensor_reduce(red[:], x2T_ps[:], mybir.AxisListType.X, A.add)
            pooledT = sb.tile([HID, 1], f32, tag="pooledT")
            nc.vector.tensor_scalar(
                pooledT[:], red[:], 1.0 / N, b2[:, 0:1], A.mult, A.add
            )
            outT_ps = ps.tile([OUT, 1], f32, tag="ps")
            nc.tensor.matmul(outT_ps[:], Wl[:], pooledT[:], start=True, stop=True)
            out_sb = sb.tile([OUT, 1], f32, tag="out_sb")
            nc.vector.tensor_scalar_add(out_sb[:], outT_ps[:], bl[:, 0:1])
            nc.sync.dma_start(out=out_d[:], in_=out_sb[:])

    _split_excess_waits(nc, mybir)
    return nc


def _prep_inputs(inputs):
    ei = np.asarray(inputs["edge_index"][0], dtype=np.int64)
    ej = np.asarray(inputs["edge_index"][1], dtype=np.int64)
    ea = np.asarray(inputs["edge_attr"], dtype=np.float32)

    ea_dense = np.zeros((N, EDIM, N), dtype=np.float32)
    ea_dense[ei, :, ej] = ea
    ea_dense[ej, :, ei] = ea
    ea_dense = np.ascontiguousarray(ea_dense.transpose(0, 1, 2).reshape(N, EDIM * N))

    svec = np.concatenate(
        [
            np.asarray(inputs["p1"], dtype=np.float32).reshape(-1),
            np.asarray(inputs["p2"], dtype=np.float32).reshape(-1),
            np.asarray(inputs["be"], dtype=np.float32).reshape(-1),
            np.asarray(inputs["We"], dtype=np.float32).reshape(-1),
        ]
    ).reshape(1, 40)

    mask = (1.0 - np.eye(N, dtype=np.float32)).astype(np.float32)

    def col(x):
        return np.ascontiguousarray(
            np.asarray(x, dtype=np.float32).reshape(-1, 1)
        )

    return {
        "ea": ea_dense,
        "encT": np.ascontiguousarray(
            np.asarray(inputs["encoding_raw"], dtype=np.float32).T
        ),
        "Wenc": np.ascontiguousarray(np.asarray(inputs["W_enc"], dtype=np.float32)),
        "W1": np.ascontiguousarray(np.asarray(inputs["W1"], dtype=np.float32)),
        "W2": np.ascontiguousarray(np.asarray(inputs["W2"], dtype=np.float32)),
        "Wl": np.ascontiguousarray(np.asarray(inputs["Wl"], dtype=np.float32)),
        "benc": col(inputs["b_enc"]),
        "b1": col(inputs["b1"]),
        "b2": col(inputs["b2"]),
        "bl": col(inputs["bl"]),
        "peT": col(inputs["pe"]),
        "svec": svec,
        "mask": mask,
    }


def kernel(**inputs) -> np.ndarray:
    import sys

    if "/opt/trn_rl_repo" not in sys.path:
        sys.path.insert(0, "/opt/trn_rl_repo")
    from concourse.bass_utils import run_bass_kernel_spmd

    if "nc" not in _CACHE:
        _CACHE["nc"] = _build()
    nc = _CACHE["nc"]

    in_map = _prep_inputs(inputs)
    res = run_bass_kernel_spmd(
        nc, [in_map] * 8, core_ids=list(range(8)), trace=False
    )
    return np.asarray(res.results[0]["out"], dtype=np.float32).reshape(1, OUT)


# revision 8
# speedup vs baseline: 1.2809x; 1.2809x over previous
"""Trainium2 Bass kernel for nn_Net_3152505995417 (gnn_message_passing).

Closed-form reformulation: with T the incidence matrix of a simple graph,
  node conv:  (T diag(d) T^T) * adj_v  ==  A with A[i,j] = d[edge(i,j)], 0 diag
  edge conv:  M = (T^T diag(dv) T) * adj_e has M[e,f] = dv[shared node],
              col-max(f=(k,l)) = max(dv[k], dv[l], 0)   (complete graph),
              row e=(i,j) of (M/colmax) @ G = dv_i*(S_i - Gn_e) + dv_j*(S_j - Gn_e)
              with Gn = G / (colmax + eps), S = T @ Gn.
So the E x E matrix is never materialized: everything lives in a dense
[N, N] node-pair layout (slots (i,j) and (j,i) both carry edge {i,j};
diagonal slots are zero). Gathers/scatters become row/column broadcasts
(PE ones-matmuls) and free-dim row-sums of [116, 116] tiles.

All inputs arrive in one packed [128, 1016] slab (two DMAs); the full
model runs replicated on each of the 8 NeuronCores (total work is a few
hundred KB — replication beats collective latency); core 0's output is
returned.
"""

import numpy as np

N = 116
E = N * (N - 1) // 2
HID = 64
EDIM = 5
OUT = 4
ENC = HID + N // 2
EPS = 1e-10

# packed slab column offsets
C_EA = 0                 # [0:116, 0:580]   ea dense, k-major (f = k*N + j)
C_SVEC = 1016            # [0, 1016:1056]   p1|p2|be|We.flat
C_ENCT = 580             # [0:122, 580:696]
C_WENC = 696             # [0:122, 696:760]
C_W1 = 760               # [0:64]
C_W2 = 824
C_WL = 888               # [0:64, 888:892]
C_MASK = 892             # [0:116, 892:1008]
C_BENC = 1008
C_B1 = 1009
C_B2 = 1010
C_PET = 1011
C_BL = 1012              # [0:4]
SLAB_W = 1056
SPLIT = 580              # DMA A = cols [0:580], DMA B = cols [580:1016]

# plane-boundary chunking of the 5*116 edge-conv slab (PE N<=512, PSUM bank)
CH = [(0, 232), (232, 580)]

_CACHE = {}


def _split_excess_waits(nc, mybir, max_waits=1):
    """Workaround: this walrus build accepts only one sync-wait per
    instruction (setupSyncWait: "Too many sync wait commands"). Move excess
    waits onto chained NoOps on the same engine immediately before the
    instruction; sequencer semantics are unchanged."""
    for fn in nc.m.functions:
        for blk in fn.blocks:
            insts = blk.instructions
            new, changed = [], False
            for ins in insts:
                si = ins.sync_info
                waits = list(si.on_wait) if si is not None else []
                if len(waits) > max_waits:
                    while len(waits) > max_waits:
                        chunk, waits = waits[:1], waits[1:]
                        nop = mybir.InstNoOp(
                            name=nc.get_next_instruction_name(),
                            engine=ins.engine,
                            sync_info=mybir.SyncInfo(on_wait=chunk, on_update=[]),
                            bass_nofuse=True,
                        )
                        new.append(nop)
                    si.on_wait = waits
                    changed = True
                new.append(ins)
            if changed:
                blk.instructions = new


def _build():
    import concourse.bass as bass
    import concourse.tile as tile
    from concourse import mybir

    f32 = mybir.dt.float32
    A = mybir.AluOpType
    Relu = mybir.ActivationFunctionType.Relu

    nc = bass.Bass("TRN2", target_bir_lowering=False, num_devices=8)

    slabA_d = nc.declare_dram_parameter("slabA", [128, SPLIT], f32, isOutput=False)
    slabB_d = nc.declare_dram_parameter(
        "slabB", [128, SLAB_W - SPLIT], f32, isOutput=False
    )
    out_d = nc.declare_dram_parameter("out", [OUT, 1], f32, isOutput=True)

    with tile.TileContext(nc) as tc:
        with (
            tc.tile_pool(name="sb", bufs=1) as sb,
            tc.tile_pool(name="pm", bufs=2) as pm,
            tc.tile_pool(name="ps", bufs=3, space="PSUM") as ps,
            tc.tile_pool(name="ps2", bufs=2, space="PSUM") as ps2,
        ):
            slab = sb.tile([128, SLAB_W], f32, tag="slab")
            nc.sync.dma_start(out=slab[:, 0:SPLIT], in_=slabA_d[:])
            nc.sync.dma_start(out=slab[:, SPLIT:SLAB_W], in_=slabB_d[:])

            ea = slab[0:N, 0:EDIM * N]
            svec = slab[0:1, C_SVEC:C_SVEC + 40]
            encT = slab[0:ENC, C_ENCT:C_ENCT + N]
            Wenc = slab[0:ENC, C_WENC:C_WENC + HID]
            W1 = slab[0:HID, C_W1:C_W1 + HID]
            W2 = slab[0:HID, C_W2:C_W2 + HID]
            Wl = slab[0:HID, C_WL:C_WL + OUT]
            mask = slab[0:N, C_MASK:C_MASK + N]
            benc = slab[0:HID, C_BENC:C_BENC + 1]
            b1 = slab[0:HID, C_B1:C_B1 + 1]
            b2 = slab[0:HID, C_B2:C_B2 + 1]
            peT = slab[0:HID, C_PET:C_PET + 1]
            bl = slab[0:OUT, C_BL:C_BL + 1]

            ones_row = sb.tile([1, N], f32, tag="ones_row")
            nc.vector.memset(ones_row[:], 1.0)
            ones_col = sb.tile([N, 1], f32, tag="ones_col")
            nc.vector.memset(ones_col[:], 1.0)

            # ---- broadcast the small row-vector params to all partitions ----
            svecB_ps = ps.tile([N, 40], f32, tag="ps")
            nc.tensor.matmul(svecB_ps[:], ones_row[:], svec, start=True, stop=True)
            svecB = sb.tile([N, 40], f32, tag="svecB")
            nc.vector.tensor_copy(svecB[:], svecB_ps[:])
            p1B = svecB[:, 0:5]
            p2B = svecB[:, 5:10]
            beB = svecB[:, 10:15]
            # We[k, m] at column 15 + k*5 + m

            # ---- x = enc @ W_enc + b_enc  (kept transposed: [HID, N]) ----
            xT_ps = ps.tile([HID, N], f32, tag="ps")
            nc.tensor.matmul(xT_ps[:], Wenc, encT, start=True, stop=True)
            xT = sb.tile([HID, N], f32, tag="xT")
            nc.vector.tensor_scalar_add(xT[:], xT_ps[:], benc)

            # ---- A1 = d1 (dense pair layout; diag slots already zero) ----
            d1 = sb.tile([N, N], f32, tag="d1")
            nc.vector.tensor_scalar_mul(d1[:], ea[:, 0:N], p1B[:, 0:1])
            for k in range(1, EDIM):
                nc.vector.scalar_tensor_tensor(
                    d1[:], ea[:, k * N:(k + 1) * N], p1B[:, k:k + 1], d1[:],
                    A.mult, A.add,
                )

            # ---- node conv 1: x1T = relu((A1 @ (x @ W1) + b1)^T) ----
            xW1_ps = ps.tile([N, HID], f32, tag="ps")
            nc.tensor.matmul(xW1_ps[:], xT[:], W1, start=True, stop=True)
            xW1 = sb.tile([N, HID], f32, tag="xW1")
            nc.vector.tensor_copy(xW1[:], xW1_ps[:])
            x1T_ps = ps.tile([HID, N], f32, tag="ps")
            nc.tensor.matmul(x1T_ps[:], xW1[:], d1[:], start=True, stop=True)
            x1T = sb.tile([HID, N], f32, tag="x1T")
            nc.scalar.activation(x1T[:], x1T_ps[:], Relu, bias=b1)

            # ---- dv = x1 @ pe^T, as row [1,N] and column [N,1] ----
            dvr_ps = ps.tile([1, N], f32, tag="ps")
            nc.tensor.matmul(dvr_ps[:], peT, x1T[:], start=True, stop=True)
            dv_row = sb.tile([1, N], f32, tag="dv_row")
            nc.vector.tensor_copy(dv_row[:], dvr_ps[:])
            dvT_ps = ps.tile([N, 1], f32, tag="ps")
            nc.tensor.matmul(dvT_ps[:], x1T[:], peT, start=True, stop=True)
            dvT = sb.tile([N, 1], f32, tag="dvT")
            nc.vector.tensor_copy(dvT[:], dvT_ps[:])
            dvROW_ps = ps.tile([N, N], f32, tag="ps")
            nc.tensor.matmul(dvROW_ps[:], ones_row[:], dv_row[:], start=True, stop=True)
            dvROW = sb.tile([N, N], f32, tag="dvROW")
            nc.vector.tensor_copy(dvROW[:], dvROW_ps[:])

            # negsumdv[i,j] = -(dv_i + dv_j);  cmeps = max(dv_i,dv_j,0)+eps
            negsumdv = sb.tile([N, N], f32, tag="negsumdv")
            nc.vector.tensor_scalar(
                negsumdv[:], dvROW[:], dvT[:, 0:1], -1.0, A.add, A.mult
            )
            cmeps = sb.tile([N, N], f32, tag="cmeps")
            nc.vector.tensor_scalar(
                cmeps[:], dvROW[:], dvT[:, 0:1], 0.0, A.max, A.max
            )
            nc.vector.tensor_scalar_add(cmeps[:], cmeps[:], EPS)
            nc.vector.reciprocal(cmeps[:], cmeps[:])

            # ---- edge conv (plane-major slabs, f = m*N + j) ----
            eR = sb.tile([N, EDIM * N], f32, tag="eR")
            nc.scalar.activation(eR[:], ea, Relu)

            G = sb.tile([N, EDIM * N], f32, tag="G")
            for m in range(EDIM):
                Gm = G[:, m * N:(m + 1) * N]
                nc.vector.tensor_scalar_mul(
                    Gm, eR[:, 0:N], svecB[:, 15 + m:15 + m + 1]
                )
                for k in range(1, EDIM):
                    nc.vector.scalar_tensor_tensor(
                        Gm, eR[:, k * N:(k + 1) * N],
                        svecB[:, 15 + k * 5 + m:15 + k * 5 + m + 1], Gm,
                        A.mult, A.add,
                    )

            # Gn = G / cmeps (per-plane broadcast of cmeps along m)
            Gn = sb.tile([N, EDIM * N], f32, tag="Gn")
            nc.vector.tensor_tensor(
                Gn[:].rearrange("p (m j) -> p m j", m=EDIM),
                G[:].rearrange("p (m j) -> p m j", m=EDIM),
                cmeps[:, None, :].to_broadcast([N, EDIM, N]),
                A.mult,
            )
            # S[i, m] = sum_j Gn_m[i, j]
            S_all = sb.tile([N, EDIM], f32, tag="S_all")
            nc.vector.tensor_reduce(
                S_all[:], Gn[:].rearrange("p (m j) -> p m j", m=EDIM),
                mybir.AxisListType.X, A.add,
            )
            # t1b[:, m] = dv_i * S[i, m] + be_m
            t1b = sb.tile([N, EDIM], f32, tag="t1b")
            nc.vector.scalar_tensor_tensor(
                t1b[:], S_all[:], dvT[:, 0:1], beB, A.mult, A.add
            )

            # q = Gn * negsumdv (broadcast along m) -- on GpSimd (idle engine)
            q = sb.tile([N, EDIM * N], f32, tag="q")
            nc.gpsimd.tensor_tensor(
                q[:].rearrange("p (m j) -> p m j", m=EDIM),
                Gn[:].rearrange("p (m j) -> p m j", m=EDIM),
                negsumdv[:, None, :].to_broadcast([N, EDIM, N]),
                A.mult,
            )

            # S as rows (colsum of symmetric Gn), then U[i, (m,j)] = dv_j*S[j,m]
            z = sb.tile([N, EDIM * N], f32, tag="z")
            for c0, c1 in CH:
                w = c1 - c0
                nm = w // N
                Srow_ps = ps2.tile([1, w], f32, tag="psrow")
                nc.tensor.matmul(
                    Srow_ps[:], ones_col[:], Gn[:, c0:c1], start=True, stop=True
                )
                u = pm.tile([1, w], f32, tag="u")
                nc.vector.tensor_tensor(
                    u[:].rearrange("p (m j) -> p m j", m=nm),
                    dv_row[:, None, :].to_broadcast([1, nm, N]),
                    Srow_ps[:].rearrange("p (m j) -> p m j", m=nm),
                    A.mult,
                )
                U_ps = ps2.tile([N, w], f32, tag="psU")
                nc.tensor.matmul(U_ps[:], ones_row[:], u[:], start=True, stop=True)
                nc.vector.tensor_tensor(z[:, c0:c1], q[:, c0:c1], U_ps[:], A.add)

            # e2_m = relu(z_m + t1b_m)
            e2 = sb.tile([N, EDIM * N], f32, tag="e2")
            for m in range(EDIM):
                nc.scalar.activation(
                    e2[:, m * N:(m + 1) * N], z[:, m * N:(m + 1) * N], Relu,
                    bias=t1b[:, m:m + 1],
                )

            # ---- A2 = (e2 @ p2^T) * mask ----
            d2 = sb.tile([N, N], f32, tag="d2")
            nc.vector.tensor_scalar_mul(d2[:], e2[:, 0:N], p2B[:, 0:1])
            for m in range(1, EDIM):
                nc.vector.scalar_tensor_tensor(
                    d2[:], e2[:, m * N:(m + 1) * N], p2B[:, m:m + 1], d2[:],
                    A.mult, A.add,
                )
            A2 = sb.tile([N, N], f32, tag="A2")
            nc.gpsimd.tensor_tensor(A2[:], d2[:], mask, A.mult)

            # ---- node conv 2 (no relu) + mean pool + head ----
            xW2_ps = ps.tile([N, HID], f32, tag="ps")
            nc.tensor.matmul(xW2_ps[:], x1T[:], W2, start=True, stop=True)
            xW2 = sb.tile([N, HID], f32, tag="xW2")
            nc.vector.tensor_copy(xW2[:], xW2_ps[:])
            x2T_ps = ps.tile([HID, N], f32, tag="ps")
            nc.tensor.matmul(x2T_ps[:], xW2[:], A2[:], start=True, stop=True)
            red = sb.tile([HID, 1], f32, tag="red")
            nc.vector.tensor_reduce(red[:], x2T_ps[:], mybir.AxisListType.X, A.add)
            pooledT = sb.tile([HID, 1], f32, tag="pooledT")
            nc.vector.tensor_scalar(
                pooledT[:], red[:], 1.0 / N, b2, A.mult, A.add
            )
            outT_ps = ps.tile([OUT, 1], f32, tag="ps")
            nc.tensor.matmul(outT_ps[:], Wl, pooledT[:], start=True, stop=True)
            out_sb = sb.tile([OUT, 1], f32, tag="out_sb")
            nc.vector.tensor_scalar_add(out_sb[:], outT_ps[:], bl)
            nc.sync.dma_start(out=out_d[:], in_=out_sb[:])

    _split_excess_waits(nc, mybir)
    return nc


def _prep_inputs(inputs):
    ei = np.asarray(inputs["edge_index"][0], dtype=np.int64)
    ej = np.asarray(inputs["edge_index"][1], dtype=np.int64)
    ea = np.asarray(inputs["edge_attr"], dtype=np.float32)

    ea_dense = np.zeros((N, EDIM, N), dtype=np.float32)
    ea_dense[ei, :, ej] = ea
    ea_dense[ej, :, ei] = ea

    slab = np.zeros((128, SLAB_W), dtype=np.float32)
    slab[0:N, 0:EDIM * N] = ea_dense.reshape(N, EDIM * N)
    slab[0, C_SVEC:C_SVEC + 40] = np.concatenate(
        [
            np.asarray(inputs["p1"], dtype=np.float32).reshape(-1),
            np.asarray(inputs["p2"], dtype=np.float32).reshape(-1),
            np.asarray(inputs["be"], dtype=np.float32).reshape(-1),
            np.asarray(inputs["We"], dtype=np.float32).reshape(-1),
        ]
    )
    slab[0:ENC, C_ENCT:C_ENCT + N] = np.asarray(
        inputs["encoding_raw"], dtype=np.float32
    ).T
    slab[0:ENC, C_WENC:C_WENC + HID] = np.asarray(inputs["W_enc"], dtype=np.float32)
    slab[0:HID, C_W1:C_W1 + HID] = np.asarray(inputs["W1"], dtype=np.float32)
    slab[0:HID, C_W2:C_W2 + HID] = np.asarray(inputs["W2"], dtype=np.float32)
    slab[0:HID, C_WL:C_WL + OUT] = np.asarray(inputs["Wl"], dtype=np.float32)
    slab[0:N, C_MASK:C_MASK + N] = 1.0 - np.eye(N, dtype=np.float32)
    slab[0:HID, C_BENC] = np.asarray(inputs["b_enc"], dtype=np.float32).reshape(-1)
    slab[0:HID, C_B1] = np.asarray(inputs["b1"], dtype=np.float32).reshape(-1)
    slab[0:HID, C_B2] = np.asarray(inputs["b2"], dtype=np.float32).reshape(-1)
    slab[0:HID, C_PET] = np.asarray(inputs["pe"], dtype=np.float32).reshape(-1)
    slab[0:OUT, C_BL] = np.asarray(inputs["bl"], dtype=np.float32).reshape(-1)

    return {
        "slabA": np.ascontiguousarray(slab[:, 0:SPLIT]),
        "slabB": np.ascontiguousarray(slab[:, SPLIT:SLAB_W]),
    }


def kernel(**inputs) -> np.ndarray:
    import sys

    if "/opt/trn_rl_repo" not in sys.path:
        sys.path.insert(0, "/opt/trn_rl_repo")
    from concourse.bass_utils import run_bass_kernel_spmd

    if "nc" not in _CACHE:
        _CACHE["nc"] = _build()
    nc = _CACHE["nc"]

    in_map = _prep_inputs(inputs)
    res = run_bass_kernel_spmd(
        nc, [in_map] * 8, core_ids=list(range(8)), trace=False
    )
    return np.asarray(res.results[0]["out"], dtype=np.float32).reshape(1, OUT)
